# revision 19
# baseline (speedup 1.0000x reference)
"""ACMIL gated-attention MIL kernel for 8 Trainium2 NeuronCores.

Strategy: data-parallel over the slide axis (B=8 -> one slide per core).
Per core:
  phase 1 (bulk, bf16): h = relu(x@W_dr), gated scores A = (tanh(h@W_v) *
    sigmoid(h@W_u)) @ W_a, streamed over 64 n-blocks of 128 patches with
    on-chip PE transposes. A is kept in a "hier" layout [128, K, 64] where
    slot (p, k, j) holds head k's score of patch n = p*64+j.
  peel: 16 rounds of global argmax per head (reduce + partition_all_reduce)
    produce the top-16 candidate positions per head in bf16 order.
  exact: the 80 candidate patch rows are re-gathered from DRAM (indirect DMA)
    and pushed through the whole chain again in exact fp32; candidates are
    ranked exactly, and ranks hit by rand_idx are dropped (mask positions
    match the fp32 reference exactly; min top-11 gap on this data is 3.5e-5
    vs ~3e-7 fp32 recompute error).
  outputs: masked A, softmax-pooled afeat via matmul (unmasked sums minus
    the 6 dropped terms per head), classifier heads.
"""
import sys

sys.path.insert(0, "/opt/trn_rl_repo")

import numpy as np

import concourse.bacc as bacc
import concourse.bass as bass
import concourse.bass_isa as bass_isa
import concourse.mybir as mybir
import concourse.tile as tile
from concourse.bass_utils import run_bass_kernel_spmd

F32 = mybir.dt.float32
BF16 = mybir.dt.bfloat16
I32 = mybir.dt.int32
I16 = mybir.dt.int16
AF = mybir.ActivationFunctionType
ALU = mybir.AluOpType
AX = mybir.AxisListType

B, N, DF, DI, DA, K = 8, 8192, 1024, 512, 128, 5
NB = N // 128          # 64 n-blocks
C = 16                 # candidates per head
NC_CAND = K * C        # 80 gathered rows
NEG = -1e9
TRACE = False

_compiled = None


def _build():
    nc = bacc.Bacc()
    x_ext = nc.declare_dram_parameter("x", [N, DF], F32, isOutput=False)
    wdr_ext = nc.declare_dram_parameter("W_dr", [DF, DI], F32, isOutput=False)
    wv_ext = nc.declare_dram_parameter("W_v", [DI, DA], F32, isOutput=False)
    wu_ext = nc.declare_dram_parameter("W_u", [DI, DA], F32, isOutput=False)
    wa_ext = nc.declare_dram_parameter("W_a", [DA, K], F32, isOutput=False)
    wcls_ext = nc.declare_dram_parameter("W_cls", [K, DI, 2], F32, isOutput=False)
    wsl_ext = nc.declare_dram_parameter("W_slide", [DI, 2], F32, isOutput=False)
    rand_ext = nc.declare_dram_parameter("rand_idx", [6], I32, isOutput=False)

    a_ext = nc.declare_dram_parameter("A_out", [N, K], F32, isOutput=True)
    br_ext = nc.declare_dram_parameter("br", [K, 2], F32, isOutput=True)
    so_ext = nc.declare_dram_parameter("so", [1, 2], F32, isOutput=True)
    sf_ext = nc.declare_dram_parameter("sf", [1, DI], F32, isOutput=True)

    with tile.TileContext(nc) as tc:
        with tc.tile_pool(name="persist", bufs=1) as pp, \
             tc.tile_pool(name="work", bufs=2) as wp, \
             tc.tile_pool(name="xin", bufs=4) as xp, \
             tc.tile_pool(name="work3", bufs=3) as w3:

            # ---------------- static prep ----------------
            # identity matrices for PE transpose
            idio = pp.tile([128, 128], I32, tag="idio")
            nc.gpsimd.iota(idio[:], pattern=[[1, 128]], base=0, channel_multiplier=-1)
            id_f32 = pp.tile([128, 128], F32, tag="idf")
            nc.vector.tensor_scalar(out=id_f32[:], in0=idio[:], scalar1=0,
                                    scalar2=None, op0=ALU.is_equal)
            id_bf = pp.tile([128, 128], BF16, tag="idb")
            nc.vector.tensor_copy(id_bf[:], id_f32[:])
            ones1 = pp.tile([1, 128], F32, tag="ones1")
            nc.vector.memset(ones1[:], 1.0)
            ones128 = pp.tile([128, 1], BF16, tag="ones128")
            nc.vector.memset(ones128[:], 1.0)
            # iota over hier slots: value = p*64 + j + 1
            iota_h_i = pp.tile([128, K, 64], I32, tag="iotahi")
            nc.gpsimd.iota(iota_h_i[:], pattern=[[0, K], [1, 64]], base=1,
                           channel_multiplier=64)
            iota_h = pp.tile([128, K, 64], F32, tag="iotah")
            nc.vector.tensor_copy(iota_h[:], iota_h_i[:])
            # headsel[p, k] = 1 iff p//16 == k  (cand partition p = k*16+j)
            hs_i = pp.tile([128, K], I32, tag="hsi")
            nc.gpsimd.iota(hs_i[:], pattern=[[-16, K]], base=0, channel_multiplier=1)
            hs_ge = pp.tile([128, K], F32, tag="hsge")
            nc.vector.tensor_scalar(out=hs_ge[:], in0=hs_i[:], scalar1=0,
                                    scalar2=None, op0=ALU.is_ge)
            hs_le = pp.tile([128, K], F32, tag="hsle")
            nc.vector.tensor_scalar(out=hs_le[:], in0=hs_i[:], scalar1=15,
                                    scalar2=None, op0=ALU.is_le)
            headsel = pp.tile([128, K], F32, tag="headsel")
            nc.vector.tensor_tensor(out=headsel[:], in0=hs_ge[:], in1=hs_le[:],
                                    op=ALU.mult)
            neg_t = pp.tile([128, K, 64], F32, tag="negt")
            nc.vector.memset(neg_t[:], NEG)

            # weights
            wdr_f = pp.tile([128, 8, DI], F32, tag="wdrf")
            nc.sync.dma_start(wdr_f[:], wdr_ext.ap().rearrange("(b p) d -> p b d", p=128))
            wdr_b = pp.tile([128, 8, DI], BF16, tag="wdrb")
            nc.vector.tensor_copy(wdr_b[:], wdr_f[:])
            wv_f = pp.tile([128, 4, DA], F32, tag="wvf")
            nc.sync.dma_start(wv_f[:], wv_ext.ap().rearrange("(b p) d -> p b d", p=128))
            wv_b = pp.tile([128, 4, DA], BF16, tag="wvb")
            nc.vector.tensor_copy(wv_b[:], wv_f[:])
            wu_f = pp.tile([128, 4, DA], F32, tag="wuf")
            nc.sync.dma_start(wu_f[:], wu_ext.ap().rearrange("(b p) d -> p b d", p=128))
            wu_b = pp.tile([128, 4, DA], BF16, tag="wub")
            nc.vector.tensor_copy(wu_b[:], wu_f[:])
            wa_f = pp.tile([128, K], F32, tag="waf")
            nc.sync.dma_start(wa_f[:], wa_ext[:])
            wa_b = pp.tile([128, K], BF16, tag="wab")
            nc.vector.tensor_copy(wa_b[:], wa_f[:])
            wcls = pp.tile([128, K, 4, 2], F32, tag="wcls")
            nc.sync.dma_start(wcls[:], wcls_ext.ap().rearrange("k (b p) c -> p k b c", p=128))
            wsl = pp.tile([128, 4, 2], F32, tag="wsl")
            nc.sync.dma_start(wsl[:], wsl_ext.ap().rearrange("(b p) c -> p b c", p=128))
            rand_i = pp.tile([1, 6], I32, tag="randi")
            nc.sync.dma_start(rand_i[:], rand_ext.ap().unsqueeze(0))
            rand_f1 = pp.tile([1, 6], F32, tag="randf1")
            nc.vector.tensor_copy(rand_f1[:], rand_i[:])

            # persistent big tensors
            h_bf = pp.tile([128, NB, DI], BF16, tag="hbf")
            a_hier = pp.tile([128, K, 64], F32, tag="ahier")

            x_view = x_ext.ap().rearrange("(p j) f -> j p f", p=128)

            # ---------------- phase 1 ----------------
            with tc.tile_pool(name="ps_t", bufs=2, space="PSUM") as ps_t, \
                 tc.tile_pool(name="ps_h", bufs=2, space="PSUM") as ps_h, \
                 tc.tile_pool(name="ps_g", bufs=1, space="PSUM") as ps_g, \
                 tc.tile_pool(name="ps_s", bufs=1, space="PSUM") as ps_s:
                gate4 = None
                for j in range(NB):
                    if j % 2 == 0:
                        # SWDGE DMA casts fp32->bf16 in flight (HWDGE can't
                        # cast); two n-blocks per transfer
                        xbf2 = xp.tile([128, 2, DF], BF16, tag="xbf")
                        nc.gpsimd.dma_start(
                            xbf2[:], x_view[j:j + 2].transpose([1, 0, 2]))
                    xbf = xbf2[:, j % 2, :]
                    # transpose x: 8 blocks of [128,128] -> xT [128f, 8, 128n]
                    # all 8 land in ONE psum tile ([128,1024] bf16 = exactly
                    # one 2KB bank) so a single copy drains them
                    xt_sb = w3.tile([128, 8, 128], BF16, tag="xtsb")
                    tpx = ps_t.tile([128, 1024], BF16, tag="tps")
                    for blk in range(8):
                        nc.tensor.transpose(
                            tpx[:, blk * 128:(blk + 1) * 128],
                            xbf[:, blk * 128:(blk + 1) * 128], id_bf[:])
                    nc.scalar.activation(
                        xt_sb[:], tpx[:].rearrange("p (b n) -> p b n", b=8),
                        AF.Copy)
                    # h = relu(x @ W_dr)
                    hp = ps_h.tile([128, DI], F32, tag="hps")
                    for fb in range(8):
                        nc.tensor.matmul(hp[:], xt_sb[:, fb, :], wdr_b[:, fb, :],
                                         start=(fb == 0), stop=(fb == 7))
                    nc.scalar.activation(h_bf[:, j, :], hp[:], AF.Relu)
                    # hT: 4 blocks [128di, 128n] into a 4-block group tile
                    if j % 4 == 0:
                        ht4 = w3.tile([128, 4, 4, 128], BF16, tag="ht4")
                    tp2 = ps_t.tile([128, 512], BF16, tag="tps")
                    for t in range(4):
                        nc.tensor.transpose(
                            tp2[:, t * 128:(t + 1) * 128],
                            h_bf[:, j, t * 128:(t + 1) * 128], id_bf[:])
                    nc.scalar.activation(
                        ht4[:, :, j % 4, :],
                        tp2[:].rearrange("p (d n) -> p d n", d=4), AF.Copy)
                    if j % 4 == 3:
                        # gates batched over 4 blocks: free dim 512
                        vps = ps_g.tile([128, 1024], F32, tag="vups")
                        for db in range(4):
                            nc.tensor.matmul(vps[:, 0:512], wv_b[:, db, :],
                                             ht4[:, db, :, :],
                                             start=(db == 0), stop=(db == 3))
                        for db in range(4):
                            nc.tensor.matmul(vps[:, 512:1024], wu_b[:, db, :],
                                             ht4[:, db, :, :],
                                             start=(db == 0), stop=(db == 3))
                        tv = wp.tile([128, 512], BF16, tag="tv")
                        nc.scalar.activation(tv[:], vps[:, 0:512], AF.Tanh)
                        tu = wp.tile([128, 512], BF16, tag="tu")
                        nc.scalar.activation(tu[:], vps[:, 512:1024], AF.Sigmoid)
                        gate4 = wp.tile([128, 512], BF16, tag="gate4")
                        nc.vector.tensor_tensor(out=gate4[:], in0=tv[:],
                                                in1=tu[:], op=ALU.mult)
                        aps = ps_s.tile([5, 512], F32, tag="aps")
                        nc.tensor.matmul(aps[:], wa_b[:], gate4[:],
                                         start=True, stop=True)
                        a5 = wp.tile([5, 512], F32, tag="a5")
                        nc.scalar.activation(a5[:], aps[:], AF.Copy)
                        atp = ps_s.tile([128, 4, 5], F32, tag="atps")
                        for t in range(4):
                            nc.tensor.transpose(
                                atp[:, t, :], a5[:, t * 128:(t + 1) * 128],
                                id_f32[0:5, 0:5])
                        nc.vector.tensor_copy(
                            a_hier[:, :, j - 3:j + 1],
                            atp[:].transpose([0, 2, 1]))

            # ---------------- peel: top-16 candidates per head ----------------
            aw = pp.tile([128, K, 64], F32, tag="aw")
            nc.vector.tensor_copy(aw[:], a_hier[:])
            candmap = pp.tile([128, K, 64], F32, tag="candmap")
            nc.vector.memset(candmap[:], 0.0)
            icoll = pp.tile([1, K, C], F32, tag="icoll")
            for r in range(C):
                pmax = wp.tile([128, K], F32, tag="pmax")
                nc.vector.tensor_reduce(pmax[:], aw[:], axis=AX.X, op=ALU.max)
                gmax = wp.tile([128, K], F32, tag="gmax")
                nc.gpsimd.partition_all_reduce(gmax[:], pmax[:], channels=128,
                                               reduce_op=bass_isa.ReduceOp.max)
                oh = wp.tile([128, K, 64], F32, tag="oh")
                nc.vector.tensor_tensor(
                    out=oh[:], in0=aw[:],
                    in1=gmax[:].unsqueeze(2).broadcast_to([128, K, 64]),
                    op=ALU.is_equal)
                nc.vector.scalar_tensor_tensor(
                    out=candmap[:], in0=oh[:], scalar=float(r + 1),
                    in1=candmap[:], op0=ALU.mult, op1=ALU.add)
                nc.vector.scalar_tensor_tensor(
                    out=aw[:], in0=oh[:], scalar=-8.0,
                    in1=aw[:], op0=ALU.mult, op1=ALU.add)
                ohm = wp.tile([128, K, 64], F32, tag="ohm")
                nc.vector.tensor_tensor(out=ohm[:], in0=oh[:], in1=iota_h[:],
                                        op=ALU.mult)
                ohi = wp.tile([128, K], F32, tag="ohi")
                nc.vector.tensor_reduce(ohi[:], ohm[:], axis=AX.X, op=ALU.add)
                gidx = wp.tile([128, K], F32, tag="gidx")
                nc.gpsimd.partition_all_reduce(gidx[:], ohi[:], channels=128,
                                               reduce_op=bass_isa.ReduceOp.add)
                nc.vector.tensor_copy(icoll[:, :, r], gidx[0:1, :])

            # per-partition candidate index tile [128,1] i32 via DRAM bounce
            idx_scr = nc.dram_tensor("idx_scratch", [C * K], F32)
            nc.sync.dma_start(idx_scr.ap().unsqueeze(0),
                              icoll[:].rearrange("o k c -> o (k c)"))
            idx_f = pp.tile([128, 1], F32, tag="idxf")
            nc.vector.memset(idx_f[:], 1.0)
            nc.sync.dma_start(idx_f[0:NC_CAND, :],
                              idx_scr.ap().rearrange("(i o) -> i o", o=1))
            idx128 = pp.tile([128, 1], I32, tag="idx128")
            nc.vector.tensor_scalar(out=idx128[:], in0=idx_f[:], scalar1=-1.0,
                                    scalar2=None, op0=ALU.add)

            # ---------------- phase 3 (unmasked sums; overlaps peel) ----------
            p_hier = pp.tile([128, K, 64], BF16, tag="phier")
            nc.scalar.activation(p_hier[:], a_hier[:], AF.Exp)
            psum_k = wp.tile([128, K], F32, tag="psumk")
            nc.vector.tensor_reduce(psum_k[:], p_hier[:], axis=AX.X, op=ALU.add)
            s_all = wp.tile([128, K], F32, tag="sall")
            nc.gpsimd.partition_all_reduce(s_all[:], psum_k[:], channels=128,
                                           reduce_op=bass_isa.ReduceOp.add)

            with tc.tile_pool(name="ps_af", bufs=1, space="PSUM") as ps_af, \
                 tc.tile_pool(name="ps_c", bufs=1, space="PSUM") as ps_c:
                afps = ps_af.tile([5, DI], F32, tag="afps")
                for j in range(NB):
                    nc.tensor.matmul(afps[:], p_hier[:, :, j], h_bf[:, j, :],
                                     start=(j == 0), stop=(j == NB - 1))

                # ---------------- exact candidate recompute ----------------
                xg = pp.tile([128, DF], F32, tag="xg")
                nc.gpsimd.indirect_dma_start(
                    out=xg[:], out_offset=None, in_=x_ext[:],
                    in_offset=bass.IndirectOffsetOnAxis(ap=idx128[:, :1], axis=0))
                xgt = wp.tile([128, 8, 128], F32, tag="xgt")
                for half in range(2):
                    tp = ps_c.tile([128, 512], F32, tag="ctps")
                    for t in range(4):
                        blk = half * 4 + t
                        nc.tensor.transpose(
                            tp[:, t * 128:(t + 1) * 128],
                            xg[:, blk * 128:(blk + 1) * 128], id_f32[:])
                    nc.vector.tensor_copy(xgt[:, half * 4:(half + 1) * 4, :], tp[:])
                hcp = ps_c.tile([128, DI], F32, tag="cbig2")
                for fb in range(8):
                    nc.tensor.matmul(hcp[:], xgt[:, fb, :], wdr_f[:, fb, :],
                                     start=(fb == 0), stop=(fb == 7))
                h_cand = pp.tile([128, DI], F32, tag="hcand")
                nc.scalar.activation(h_cand[:], hcp[:], AF.Relu)
                htc = wp.tile([128, 4, 128], F32, tag="htc")
                tp3 = ps_c.tile([128, 512], F32, tag="ctps")
                for t in range(4):
                    nc.tensor.transpose(tp3[:, t * 128:(t + 1) * 128],
                                        h_cand[:, t * 128:(t + 1) * 128], id_f32[:])
                nc.vector.tensor_copy(htc[:], tp3[:])
                vcps = ps_c.tile([128, 256], F32, tag="cbig2")
                for db in range(4):
                    nc.tensor.matmul(vcps[:, 0:128], wv_f[:, db, :], htc[:, db, :],
                                     start=(db == 0), stop=(db == 3))
                for db in range(4):
                    nc.tensor.matmul(vcps[:, 128:256], wu_f[:, db, :], htc[:, db, :],
                                     start=(db == 0), stop=(db == 3))
                tvc = wp.tile([128, 128], F32, tag="tvc")
                nc.scalar.activation(tvc[:], vcps[:, 0:128], AF.Tanh)
                # exact sigmoid via exp (exp LUT ~2ulp; sigmoid LUT is 40ulp)
                enu = wp.tile([128, 128], F32, tag="enu")
                nc.scalar.activation(enu[:], vcps[:, 128:256], AF.Exp, scale=-1.0)
                den = wp.tile([128, 128], F32, tag="den")
                nc.vector.tensor_scalar(out=den[:], in0=enu[:], scalar1=1.0,
                                        scalar2=None, op0=ALU.add)
                sgc = wp.tile([128, 128], F32, tag="sgc")
                nc.vector.reciprocal(sgc[:], den[:])
                gc = wp.tile([128, 128], F32, tag="gc")
                nc.vector.tensor_tensor(out=gc[:], in0=tvc[:], in1=sgc[:],
                                        op=ALU.mult)
                acps = ps_c.tile([5, 128], F32, tag="c1")
                nc.tensor.matmul(acps[:], wa_f[:], gc[:], start=True, stop=True)
                ac5 = wp.tile([5, 128], F32, tag="ac5")
                nc.vector.tensor_copy(ac5[:], acps[:])
                actp = ps_c.tile([128, 5], F32, tag="c1")
                nc.tensor.transpose(actp[:], ac5[:], id_f32[0:5, 0:5])
                avm = wp.tile([128, K], F32, tag="avm")
                nc.vector.tensor_tensor(out=avm[:], in0=actp[:], in1=headsel[:],
                                        op=ALU.mult)
                av = pp.tile([128, 1], F32, tag="av")
                nc.vector.tensor_reduce(av[:], avm[:], axis=AX.X, op=ALU.add)

                # exact ranks: rank_i = #(same head, value greater)
                avtp = ps_c.tile([1, 128], F32, tag="c1")
                nc.tensor.transpose(avtp[:], av[:], id_f32[:])
                avt = wp.tile([1, 128], F32, tag="avt")
                nc.vector.tensor_copy(avt[:], avtp[:])
                avbp = ps_c.tile([128, 128], F32, tag="c1")
                nc.tensor.matmul(avbp[:], ones1[:], avt[:], start=True, stop=True)
                g_gt = wp.tile([128, 128], F32, tag="ggt")
                nc.vector.tensor_scalar(out=g_gt[:], in0=avbp[:], scalar1=av[:],
                                        scalar2=None, op0=ALU.is_gt)
                # same-head mask: headsel @ headsel.T via PE
                hstp = ps_c.tile([5, 128], F32, tag="c1")
                nc.tensor.transpose(hstp[:], headsel[:], id_f32[:])
                hst = wp.tile([5, 128], F32, tag="hst")
                nc.vector.tensor_copy(hst[:], hstp[:])
                hmp = ps_c.tile([128, 128], F32, tag="c1")
                nc.tensor.matmul(hmp[:], hst[:], hst[:], start=True, stop=True)
                gm = wp.tile([128, 128], F32, tag="gm")
                nc.vector.tensor_tensor(out=gm[:], in0=g_gt[:], in1=hmp[:],
                                        op=ALU.mult)
                rank = pp.tile([128, 1], F32, tag="rank")
                nc.vector.tensor_reduce(rank[:], gm[:], axis=AX.X, op=ALU.add)
                # dropflag_i = rank_i in rand_idx
                randb_ps = ps_c.tile([128, 6], F32, tag="c1")
                nc.tensor.matmul(randb_ps[:], ones1[:], rand_f1[:],
                                 start=True, stop=True)
                feq = wp.tile([128, 6], F32, tag="feq")
                nc.vector.tensor_scalar(out=feq[:], in0=randb_ps[:],
                                        scalar1=rank[:], scalar2=None,
                                        op0=ALU.is_equal)
                dropflag = pp.tile([128, 1], F32, tag="dropflag")
                nc.vector.tensor_reduce(dropflag[:], feq[:], axis=AX.X, op=ALU.add)

                # ---------------- dropmask over full N ----------------
                dftp = ps_c.tile([1, 128], F32, tag="c1")
                nc.tensor.transpose(dftp[:], dropflag[:], id_f32[:])
                dft = wp.tile([1, 128], F32, tag="dft")
                nc.vector.tensor_copy(dft[:], dftp[:])
                fbp = ps_c.tile([128, 128], F32, tag="c1")
                nc.tensor.matmul(fbp[:], ones1[:], dft[:], start=True, stop=True)
                fb_sb = pp.tile([128, 128], F32, tag="fbsb")
                nc.vector.tensor_copy(fb_sb[:], fbp[:])
                fb3 = fb_sb[:, 0:NC_CAND].rearrange("p (k c) -> p k c", k=K)
                dropmask = pp.tile([128, K, 64], F32, tag="dropmask")
                nc.vector.memset(dropmask[:], 0.0)
                for r in range(C):
                    mr = wp.tile([128, K, 64], F32, tag="mr")
                    nc.vector.tensor_scalar(out=mr[:], in0=candmap[:],
                                            scalar1=float(r + 1), scalar2=None,
                                            op0=ALU.is_equal)
                    nc.vector.tensor_tensor(
                        out=mr[:], in0=mr[:],
                        in1=fb3[:, :, r].unsqueeze(2).broadcast_to([128, K, 64]),
                        op=ALU.mult)
                    nc.vector.tensor_tensor(out=dropmask[:], in0=dropmask[:],
                                            in1=mr[:], op=ALU.add)
                dm_u8 = pp.tile([128, K, 64], mybir.dt.uint8, tag="dmu8")
                nc.vector.tensor_copy(dm_u8[:], dropmask[:])
                a_msk = pp.tile([128, K, 64], F32, tag="amsk")
                nc.vector.tensor_copy(a_msk[:], a_hier[:])
                nc.vector.copy_predicated(a_msk[:], dm_u8[:], neg_t[:])
                a_out_t = pp.tile([128, 64, K], F32, tag="aoutt")
                nc.vector.tensor_copy(a_out_t[:], a_msk[:].transpose([0, 2, 1]))
                nc.sync.dma_start(
                    a_ext.ap().rearrange("(p j) k -> p j k", p=128),
                    a_out_t[:])

                # ---------------- corrections + outputs ----------------
                expv = wp.tile([128, 1], F32, tag="expv")
                nc.scalar.activation(expv[:], av[:], AF.Exp)
                wc1 = wp.tile([128, 1], F32, tag="wc1")
                nc.vector.tensor_tensor(out=wc1[:], in0=dropflag[:], in1=expv[:],
                                        op=ALU.mult)
                wcand = wp.tile([128, K], BF16, tag="wcand")
                nc.vector.tensor_scalar(out=wcand[:], in0=headsel[:],
                                        scalar1=wc1[:], scalar2=None,
                                        op0=ALU.mult)
                hcb = wp.tile([128, DI], BF16, tag="hcb")
                nc.vector.tensor_copy(hcb[:], h_cand[:])
                corr_ps = ps_c.tile([5, DI], F32, tag="corrps")
                nc.tensor.matmul(corr_ps[:], wcand[:], hcb[:], start=True, stop=True)
                cs_ps = ps_c.tile([5, 1], F32, tag="csps")
                nc.tensor.matmul(cs_ps[:], wcand[:], ones128[:], start=True, stop=True)
                # S per head as [5,1]: transpose one row of s_all
                s15 = wp.tile([1, K], F32, tag="s15")
                nc.vector.tensor_copy(s15[:], s_all[0:1, :])
                s5p = ps_c.tile([5, 1], F32, tag="s5p")
                nc.tensor.transpose(s5p[:], s15[:], id_f32[0:1, 0:1])
                cs_sb = wp.tile([5, 1], F32, tag="cssb")
                nc.vector.tensor_copy(cs_sb[:], cs_ps[:])
                smask = wp.tile([5, 1], F32, tag="smask")
                nc.vector.tensor_tensor(out=smask[:], in0=s5p[:], in1=cs_sb[:],
                                        op=ALU.subtract)
                srec = wp.tile([5, 1], F32, tag="srec")
                nc.vector.reciprocal(srec[:], smask[:])
                corr_sb = wp.tile([5, DI], F32, tag="corrsb")
                nc.vector.tensor_copy(corr_sb[:], corr_ps[:])
                afm = wp.tile([5, DI], F32, tag="afm")
                nc.vector.tensor_tensor(out=afm[:], in0=afps[:], in1=corr_sb[:],
                                        op=ALU.subtract)
                afn = wp.tile([5, DI], F32, tag="afn")
                nc.vector.tensor_scalar(out=afn[:], in0=afm[:], scalar1=srec[:],
                                        scalar2=None, op0=ALU.mult)
                afntp = ps_c.tile([128, 4, 5], F32, tag="c1")
                for t in range(4):
                    nc.tensor.transpose(afntp[:, t, :],
                                        afn[:, t * 128:(t + 1) * 128],
                                        id_f32[0:5, 0:5])
                afnt = wp.tile([128, 4, 5], F32, tag="afnt")
                nc.vector.tensor_copy(afnt[:], afntp[:])
                br_ps = ps_c.tile([1, K * 2], F32, tag="outp")
                for k in range(K):
                    for t in range(4):
                        nc.tensor.matmul(br_ps[:, 2 * k:2 * k + 2],
                                         afnt[:, t, k:k + 1], wcls[:, k, t, :],
                                         start=(t == 0), stop=(t == 3))
                br_sb = wp.tile([1, K * 2], F32, tag="brsb")
                nc.vector.tensor_copy(br_sb[:], br_ps[:])
                nc.sync.dma_start(br_ext.ap().rearrange("k c -> (k c)").unsqueeze(0),
                                  br_sb[:])
                sft = wp.tile([128, 4], F32, tag="sft")
                nc.vector.tensor_reduce(sft[:], afnt[:], axis=AX.X, op=ALU.add)
                sfs = wp.tile([128, 4], F32, tag="sfs")
                nc.vector.tensor_scalar(out=sfs[:], in0=sft[:], scalar1=1.0 / K,
                                        scalar2=None, op0=ALU.mult)
                nc.sync.dma_start(sf_ext.ap().rearrange("o (b p) -> p (o b)", p=128),
                                  sfs[:])
                so_ps = ps_c.tile([1, 2], F32, tag="outp")
                for t in range(4):
                    nc.tensor.matmul(so_ps[:], sfs[:, t:t + 1], wsl[:, t, :],
                                     start=(t == 0), stop=(t == 3))
                so_sb = wp.tile([1, 2], F32, tag="sosb")
                nc.vector.tensor_copy(so_sb[:], so_ps[:])
                nc.sync.dma_start(so_ext[:], so_sb[:])
    nc.finalize()
    return nc


def kernel(**inputs):
    global _compiled
    x = np.ascontiguousarray(np.asarray(inputs["x"], dtype=np.float32))
    names = ["W_dr", "W_v", "W_u", "W_a", "W_cls", "W_slide"]
    w = {n: np.ascontiguousarray(np.asarray(inputs[n], np.float32)) for n in names}
    rand_idx = np.ascontiguousarray(np.asarray(inputs["rand_idx"], np.int32))

    if _compiled is None:
        _compiled = _build()
    nc = _compiled

    in_maps = []
    for b in range(B):
        m = {"x": x[b], "rand_idx": rand_idx}
        m.update(w)
        in_maps.append(m)
    res = run_bass_kernel_spmd(nc, in_maps, core_ids=list(range(B)),
                               trace=TRACE)
    if TRACE:
        print("HW exec time:", res.exec_time_ns, "ns")
        print("mean exec:", res.mean_exec_time_ns,
              "max core:", res.max_exec_time_core_id)

    A = np.stack([res.results[b]["A_out"] for b in range(B)])        # (B, N, K)
    br = np.stack([res.results[b]["br"] for b in range(B)], axis=1)  # (K, B, 2)
    so = np.concatenate([res.results[b]["so"] for b in range(B)])    # (B, 2)
    sf = np.concatenate([res.results[b]["sf"] for b in range(B)])    # (B, DI)
    return (br.astype(np.float32), so.astype(np.float32),
            A[None].astype(np.float32), sf.astype(np.float32))


# revision 21
# speedup vs baseline: 1.0632x; 1.0632x over previous
"""ACMIL gated-attention MIL kernel for 8 Trainium2 NeuronCores.

Strategy: data-parallel over the slide axis (B=8 -> one slide per core).
Per core:
  phase 1 (bulk, bf16): h = relu(x@W_dr), gated scores A = (tanh(h@W_v) *
    sigmoid(h@W_u)) @ W_a, streamed over 64 n-blocks of 128 patches with
    on-chip PE transposes. A is kept in a "hier" layout [128, K, 64] where
    slot (p, k, j) holds head k's score of patch n = p*64+j.
  peel: 16 rounds of global argmax per head (reduce + partition_all_reduce)
    produce the top-16 candidate positions per head in bf16 order.
  exact: the 80 candidate patch rows are re-gathered from DRAM (indirect DMA)
    and pushed through the whole chain again in exact fp32; candidates are
    ranked exactly, and ranks hit by rand_idx are dropped (mask positions
    match the fp32 reference exactly; min top-11 gap on this data is 3.5e-5
    vs ~3e-7 fp32 recompute error).
  outputs: masked A, softmax-pooled afeat via matmul (unmasked sums minus
    the 6 dropped terms per head), classifier heads.
"""
import sys

sys.path.insert(0, "/opt/trn_rl_repo")

import numpy as np

import concourse.bacc as bacc
import concourse.bass as bass
import concourse.bass_isa as bass_isa
import concourse.mybir as mybir
import concourse.tile as tile
from concourse.bass_utils import run_bass_kernel_spmd

F32 = mybir.dt.float32
BF16 = mybir.dt.bfloat16
I32 = mybir.dt.int32
I16 = mybir.dt.int16
AF = mybir.ActivationFunctionType
ALU = mybir.AluOpType
AX = mybir.AxisListType

B, N, DF, DI, DA, K = 8, 8192, 1024, 512, 128, 5
NB = N // 128          # 64 n-blocks
C = 16                 # candidates per head
NC_CAND = K * C        # 80 gathered rows
NEG = -1e9
TRACE = False

_compiled = None


def _build():
    nc = bacc.Bacc()
    x_ext = nc.declare_dram_parameter("x", [N, DF], F32, isOutput=False)
    wdr_ext = nc.declare_dram_parameter("W_dr", [DF, DI], F32, isOutput=False)
    wv_ext = nc.declare_dram_parameter("W_v", [DI, DA], F32, isOutput=False)
    wu_ext = nc.declare_dram_parameter("W_u", [DI, DA], F32, isOutput=False)
    wa_ext = nc.declare_dram_parameter("W_a", [DA, K], F32, isOutput=False)
    wcls_ext = nc.declare_dram_parameter("W_cls", [K, DI, 2], F32, isOutput=False)
    wsl_ext = nc.declare_dram_parameter("W_slide", [DI, 2], F32, isOutput=False)
    rand_ext = nc.declare_dram_parameter("rand_idx", [6], I32, isOutput=False)

    a_ext = nc.declare_dram_parameter("A_out", [N, K], F32, isOutput=True)
    br_ext = nc.declare_dram_parameter("br", [K, 2], F32, isOutput=True)
    so_ext = nc.declare_dram_parameter("so", [1, 2], F32, isOutput=True)
    sf_ext = nc.declare_dram_parameter("sf", [1, DI], F32, isOutput=True)

    with tile.TileContext(nc) as tc:
        with tc.tile_pool(name="persist", bufs=1) as pp, \
             tc.tile_pool(name="work", bufs=2) as wp, \
             tc.tile_pool(name="xin", bufs=4) as xp, \
             tc.tile_pool(name="work3", bufs=3) as w3:

            # ---------------- static prep ----------------
            # identity matrices for PE transpose
            idio = pp.tile([128, 128], I32, tag="idio")
            nc.gpsimd.iota(idio[:], pattern=[[1, 128]], base=0, channel_multiplier=-1)
            id_f32 = pp.tile([128, 128], F32, tag="idf")
            nc.vector.tensor_scalar(out=id_f32[:], in0=idio[:], scalar1=0,
                                    scalar2=None, op0=ALU.is_equal)
            id_bf = pp.tile([128, 128], BF16, tag="idb")
            nc.vector.tensor_copy(id_bf[:], id_f32[:])
            ones1 = pp.tile([1, 128], F32, tag="ones1")
            nc.vector.memset(ones1[:], 1.0)
            ones128 = pp.tile([128, 1], BF16, tag="ones128")
            nc.vector.memset(ones128[:], 1.0)
            ones128f = pp.tile([128, 1], F32, tag="ones128f")
            nc.vector.memset(ones128f[:], 1.0)
            # iota over hier slots: value = p*64 + j + 1
            iota_h_i = pp.tile([128, K, 64], I32, tag="iotahi")
            nc.gpsimd.iota(iota_h_i[:], pattern=[[0, K], [1, 64]], base=1,
                           channel_multiplier=64)
            iota_h = pp.tile([128, K, 64], F32, tag="iotah")
            nc.vector.tensor_copy(iota_h[:], iota_h_i[:])
            # headsel[p, k] = 1 iff p//16 == k  (cand partition p = k*16+j)
            hs_i = pp.tile([128, K], I32, tag="hsi")
            nc.gpsimd.iota(hs_i[:], pattern=[[-16, K]], base=0, channel_multiplier=1)
            hs_ge = pp.tile([128, K], F32, tag="hsge")
            nc.vector.tensor_scalar(out=hs_ge[:], in0=hs_i[:], scalar1=0,
                                    scalar2=None, op0=ALU.is_ge)
            hs_le = pp.tile([128, K], F32, tag="hsle")
            nc.vector.tensor_scalar(out=hs_le[:], in0=hs_i[:], scalar1=15,
                                    scalar2=None, op0=ALU.is_le)
            headsel = pp.tile([128, K], F32, tag="headsel")
            nc.vector.tensor_tensor(out=headsel[:], in0=hs_ge[:], in1=hs_le[:],
                                    op=ALU.mult)
            neg_t = pp.tile([128, K, 64], F32, tag="negt")
            nc.vector.memset(neg_t[:], NEG)

            # weights
            wdr_f = pp.tile([128, 8, DI], F32, tag="wdrf")
            nc.sync.dma_start(wdr_f[:], wdr_ext.ap().rearrange("(b p) d -> p b d", p=128))
            wdr_b = pp.tile([128, 8, DI], BF16, tag="wdrb")
            nc.vector.tensor_copy(wdr_b[:], wdr_f[:])
            wv_f = pp.tile([128, 4, DA], F32, tag="wvf")
            nc.sync.dma_start(wv_f[:], wv_ext.ap().rearrange("(b p) d -> p b d", p=128))
            wv_b = pp.tile([128, 4, DA], BF16, tag="wvb")
            nc.vector.tensor_copy(wv_b[:], wv_f[:])
            wu_f = pp.tile([128, 4, DA], F32, tag="wuf")
            nc.sync.dma_start(wu_f[:], wu_ext.ap().rearrange("(b p) d -> p b d", p=128))
            wu_b = pp.tile([128, 4, DA], BF16, tag="wub")
            nc.vector.tensor_copy(wu_b[:], wu_f[:])
            wa_f = pp.tile([128, K], F32, tag="waf")
            nc.sync.dma_start(wa_f[:], wa_ext[:])
            wa_b = pp.tile([128, K], BF16, tag="wab")
            nc.vector.tensor_copy(wa_b[:], wa_f[:])
            wcls = pp.tile([128, K, 4, 2], F32, tag="wcls")
            nc.sync.dma_start(wcls[:], wcls_ext.ap().rearrange("k (b p) c -> p k b c", p=128))
            wsl = pp.tile([128, 4, 2], F32, tag="wsl")
            nc.sync.dma_start(wsl[:], wsl_ext.ap().rearrange("(b p) c -> p b c", p=128))
            rand_i = pp.tile([1, 6], I32, tag="randi")
            nc.sync.dma_start(rand_i[:], rand_ext.ap().unsqueeze(0))
            rand_f1 = pp.tile([1, 6], F32, tag="randf1")
            nc.vector.tensor_copy(rand_f1[:], rand_i[:])

            # persistent big tensors
            h_bf = pp.tile([128, NB, DI], BF16, tag="hbf")
            a_hier = pp.tile([128, K, 64], F32, tag="ahier")

            x_view = x_ext.ap().rearrange("(p j) f -> j p f", p=128)

            # ---------------- phase 1 ----------------
            with tc.tile_pool(name="ps_t", bufs=2, space="PSUM") as ps_t, \
                 tc.tile_pool(name="ps_h", bufs=2, space="PSUM") as ps_h, \
                 tc.tile_pool(name="ps_g", bufs=1, space="PSUM") as ps_g, \
                 tc.tile_pool(name="ps_s", bufs=1, space="PSUM") as ps_s:
                gate4 = None
                for j in range(NB):
                    if j % 2 == 0:
                        # SWDGE DMA casts fp32->bf16 in flight (HWDGE can't
                        # cast); two n-blocks per transfer
                        xbf2 = xp.tile([128, 2, DF], BF16, tag="xbf")
                        nc.gpsimd.dma_start(
                            xbf2[:], x_view[j:j + 2].transpose([1, 0, 2]))
                    xbf = xbf2[:, j % 2, :]
                    # transpose x: 8 blocks of [128,128] -> xT [128f, 8, 128n]
                    # all 8 land in ONE psum tile ([128,1024] bf16 = exactly
                    # one 2KB bank) so a single copy drains them
                    xt_sb = w3.tile([128, 8, 128], BF16, tag="xtsb")
                    tpx = ps_t.tile([128, 1024], BF16, tag="tps")
                    for blk in range(8):
                        nc.tensor.transpose(
                            tpx[:, blk * 128:(blk + 1) * 128],
                            xbf[:, blk * 128:(blk + 1) * 128], id_bf[:])
                    nc.scalar.activation(
                        xt_sb[:], tpx[:].rearrange("p (b n) -> p b n", b=8),
                        AF.Copy)
                    # h = relu(x @ W_dr)
                    hp = ps_h.tile([128, DI], F32, tag="hps")
                    for fb in range(8):
                        nc.tensor.matmul(hp[:], xt_sb[:, fb, :], wdr_b[:, fb, :],
                                         start=(fb == 0), stop=(fb == 7))
                    nc.scalar.activation(h_bf[:, j, :], hp[:], AF.Relu)
                    # hT: 4 blocks [128di, 128n] into a 4-block group tile
                    if j % 4 == 0:
                        ht4 = w3.tile([128, 4, 4, 128], BF16, tag="ht4")
                    tp2 = ps_t.tile([128, 512], BF16, tag="tps")
                    for t in range(4):
                        nc.tensor.transpose(
                            tp2[:, t * 128:(t + 1) * 128],
                            h_bf[:, j, t * 128:(t + 1) * 128], id_bf[:])
                    nc.scalar.activation(
                        ht4[:, :, j % 4, :],
                        tp2[:].rearrange("p (d n) -> p d n", d=4), AF.Copy)
                    if j % 4 == 3:
                        # gates batched over 4 blocks: free dim 512
                        vps = ps_g.tile([128, 1024], F32, tag="vups")
                        for db in range(4):
                            nc.tensor.matmul(vps[:, 0:512], wv_b[:, db, :],
                                             ht4[:, db, :, :],
                                             start=(db == 0), stop=(db == 3))
                        for db in range(4):
                            nc.tensor.matmul(vps[:, 512:1024], wu_b[:, db, :],
                                             ht4[:, db, :, :],
                                             start=(db == 0), stop=(db == 3))
                        tv = wp.tile([128, 512], BF16, tag="tv")
                        nc.scalar.activation(tv[:], vps[:, 0:512], AF.Tanh)
                        tu = wp.tile([128, 512], BF16, tag="tu")
                        nc.scalar.activation(tu[:], vps[:, 512:1024], AF.Sigmoid)
                        gate4 = wp.tile([128, 512], BF16, tag="gate4")
                        nc.vector.tensor_tensor(out=gate4[:], in0=tv[:],
                                                in1=tu[:], op=ALU.mult)
                        aps = ps_s.tile([5, 512], F32, tag="aps")
                        nc.tensor.matmul(aps[:], wa_b[:], gate4[:],
                                         start=True, stop=True)
                        a5 = wp.tile([5, 512], F32, tag="a5")
                        nc.scalar.activation(a5[:], aps[:], AF.Copy)
                        atp = ps_s.tile([128, 4, 5], F32, tag="atps")
                        for t in range(4):
                            nc.tensor.transpose(
                                atp[:, t, :], a5[:, t * 128:(t + 1) * 128],
                                id_f32[0:5, 0:5])
                        nc.vector.tensor_copy(
                            a_hier[:, :, j - 3:j + 1],
                            atp[:].transpose([0, 2, 1]))

            # ---------------- peel: top-16 candidates per head ----------------
            ps_p_cm = tc.tile_pool(name="ps_p", bufs=2, space="PSUM")
            ps_p = ps_p_cm.__enter__()
            aw = pp.tile([128, K, 64], F32, tag="aw")
            nc.vector.tensor_copy(aw[:], a_hier[:])
            candmap = pp.tile([128, K, 64], F32, tag="candmap")
            nc.vector.memset(candmap[:], 0.0)
            icoll = pp.tile([1, K, C], F32, tag="icoll")
            for r in range(C):
                pmax = wp.tile([128, K], F32, tag="pmax")
                nc.vector.tensor_reduce(pmax[:], aw[:], axis=AX.X, op=ALU.max)
                gmax = wp.tile([128, K], F32, tag="gmax")
                nc.gpsimd.partition_all_reduce(gmax[:], pmax[:], channels=128,
                                               reduce_op=bass_isa.ReduceOp.max)
                oh = wp.tile([128, K, 64], F32, tag="oh")
                nc.vector.tensor_tensor(
                    out=oh[:], in0=aw[:],
                    in1=gmax[:].unsqueeze(2).broadcast_to([128, K, 64]),
                    op=ALU.is_equal)
                nc.vector.scalar_tensor_tensor(
                    out=candmap[:], in0=oh[:], scalar=float(r + 1),
                    in1=candmap[:], op0=ALU.mult, op1=ALU.add)
                nc.vector.scalar_tensor_tensor(
                    out=aw[:], in0=oh[:], scalar=-8.0,
                    in1=aw[:], op0=ALU.mult, op1=ALU.add)
                ohm = wp.tile([128, K, 64], F32, tag="ohm")
                nc.vector.tensor_tensor(out=ohm[:], in0=oh[:], in1=iota_h[:],
                                        op=ALU.mult)
                ohi = wp.tile([128, K], F32, tag="ohi")
                nc.vector.tensor_reduce(ohi[:], ohm[:], axis=AX.X, op=ALU.add)
                # partition-sum via PE (ones-column matmul) instead of
                # the Q7 software all-reduce
                idxp = ps_p.tile([1, K], F32, tag="idxp")
                nc.tensor.matmul(idxp[:], ones128f[:], ohi[:],
                                 start=True, stop=True)
                nc.vector.tensor_copy(icoll[:, :, r], idxp[:])

            ps_p_cm.__exit__(None, None, None)
            # per-partition candidate index tile [128,1] i32 via DRAM bounce
            idx_scr = nc.dram_tensor("idx_scratch", [C * K], F32)
            nc.sync.dma_start(idx_scr.ap().unsqueeze(0),
                              icoll[:].rearrange("o k c -> o (k c)"))
            idx_f = pp.tile([128, 1], F32, tag="idxf")
            nc.vector.memset(idx_f[:], 1.0)
            nc.sync.dma_start(idx_f[0:NC_CAND, :],
                              idx_scr.ap().rearrange("(i o) -> i o", o=1))
            idx128 = pp.tile([128, 1], I32, tag="idx128")
            nc.vector.tensor_scalar(out=idx128[:], in0=idx_f[:], scalar1=-1.0,
                                    scalar2=None, op0=ALU.add)

            # ---------------- phase 3 (unmasked sums; overlaps peel) ----------
            p_hier = pp.tile([128, K, 64], BF16, tag="phier")
            nc.scalar.activation(p_hier[:], a_hier[:], AF.Exp)
            psum_k = wp.tile([128, K], F32, tag="psumk")
            nc.vector.tensor_reduce(psum_k[:], p_hier[:], axis=AX.X, op=ALU.add)
            s_all = wp.tile([128, K], F32, tag="sall")
            nc.gpsimd.partition_all_reduce(s_all[:], psum_k[:], channels=128,
                                           reduce_op=bass_isa.ReduceOp.add)

            with tc.tile_pool(name="ps_af", bufs=1, space="PSUM") as ps_af, \
                 tc.tile_pool(name="ps_c", bufs=1, space="PSUM") as ps_c:
                afps = ps_af.tile([5, DI], F32, tag="afps")
                for j in range(NB):
                    nc.tensor.matmul(afps[:], p_hier[:, :, j], h_bf[:, j, :],
                                     start=(j == 0), stop=(j == NB - 1))

                # ---------------- exact candidate recompute ----------------
                xg = pp.tile([128, DF], F32, tag="xg")
                nc.gpsimd.indirect_dma_start(
                    out=xg[:], out_offset=None, in_=x_ext[:],
                    in_offset=bass.IndirectOffsetOnAxis(ap=idx128[:, :1], axis=0))
                xgt = wp.tile([128, 8, 128], F32, tag="xgt")
                for half in range(2):
                    tp = ps_c.tile([128, 512], F32, tag="ctps")
                    for t in range(4):
                        blk = half * 4 + t
                        nc.tensor.transpose(
                            tp[:, t * 128:(t + 1) * 128],
                            xg[:, blk * 128:(blk + 1) * 128], id_f32[:])
                    nc.vector.tensor_copy(xgt[:, half * 4:(half + 1) * 4, :], tp[:])
                hcp = ps_c.tile([128, DI], F32, tag="cbig2")
                for fb in range(8):
                    nc.tensor.matmul(hcp[:], xgt[:, fb, :], wdr_f[:, fb, :],
                                     start=(fb == 0), stop=(fb == 7))
                h_cand = pp.tile([128, DI], F32, tag="hcand")
                nc.scalar.activation(h_cand[:], hcp[:], AF.Relu)
                htc = wp.tile([128, 4, 128], F32, tag="htc")
                tp3 = ps_c.tile([128, 512], F32, tag="ctps")
                for t in range(4):
                    nc.tensor.transpose(tp3[:, t * 128:(t + 1) * 128],
                                        h_cand[:, t * 128:(t + 1) * 128], id_f32[:])
                nc.vector.tensor_copy(htc[:], tp3[:])
                vcps = ps_c.tile([128, 256], F32, tag="cbig2")
                for db in range(4):
                    nc.tensor.matmul(vcps[:, 0:128], wv_f[:, db, :], htc[:, db, :],
                                     start=(db == 0), stop=(db == 3))
                for db in range(4):
                    nc.tensor.matmul(vcps[:, 128:256], wu_f[:, db, :], htc[:, db, :],
                                     start=(db == 0), stop=(db == 3))
                tvc = wp.tile([128, 128], F32, tag="tvc")
                nc.scalar.activation(tvc[:], vcps[:, 0:128], AF.Tanh)
                # exact sigmoid via exp (exp LUT ~2ulp; sigmoid LUT is 40ulp)
                enu = wp.tile([128, 128], F32, tag="enu")
                nc.scalar.activation(enu[:], vcps[:, 128:256], AF.Exp, scale=-1.0)
                den = wp.tile([128, 128], F32, tag="den")
                nc.vector.tensor_scalar(out=den[:], in0=enu[:], scalar1=1.0,
                                        scalar2=None, op0=ALU.add)
                sgc = wp.tile([128, 128], F32, tag="sgc")
                nc.vector.reciprocal(sgc[:], den[:])
                gc = wp.tile([128, 128], F32, tag="gc")
                nc.vector.tensor_tensor(out=gc[:], in0=tvc[:], in1=sgc[:],
                                        op=ALU.mult)
                acps = ps_c.tile([5, 128], F32, tag="c1")
                nc.tensor.matmul(acps[:], wa_f[:], gc[:], start=True, stop=True)
                ac5 = wp.tile([5, 128], F32, tag="ac5")
                nc.vector.tensor_copy(ac5[:], acps[:])
                actp = ps_c.tile([128, 5], F32, tag="c1")
                nc.tensor.transpose(actp[:], ac5[:], id_f32[0:5, 0:5])
                avm = wp.tile([128, K], F32, tag="avm")
                nc.vector.tensor_tensor(out=avm[:], in0=actp[:], in1=headsel[:],
                                        op=ALU.mult)
                av = pp.tile([128, 1], F32, tag="av")
                nc.vector.tensor_reduce(av[:], avm[:], axis=AX.X, op=ALU.add)

                # exact ranks: rank_i = #(same head, value greater)
                avtp = ps_c.tile([1, 128], F32, tag="c1")
                nc.tensor.transpose(avtp[:], av[:], id_f32[:])
                avt = wp.tile([1, 128], F32, tag="avt")
                nc.vector.tensor_copy(avt[:], avtp[:])
                avbp = ps_c.tile([128, 128], F32, tag="c1")
                nc.tensor.matmul(avbp[:], ones1[:], avt[:], start=True, stop=True)
                g_gt = wp.tile([128, 128], F32, tag="ggt")
                nc.vector.tensor_scalar(out=g_gt[:], in0=avbp[:], scalar1=av[:],
                                        scalar2=None, op0=ALU.is_gt)
                # same-head mask: headsel @ headsel.T via PE
                hstp = ps_c.tile([5, 128], F32, tag="c1")
                nc.tensor.transpose(hstp[:], headsel[:], id_f32[:])
                hst = wp.tile([5, 128], F32, tag="hst")
                nc.vector.tensor_copy(hst[:], hstp[:])
                hmp = ps_c.tile([128, 128], F32, tag="c1")
                nc.tensor.matmul(hmp[:], hst[:], hst[:], start=True, stop=True)
                gm = wp.tile([128, 128], F32, tag="gm")
                nc.vector.tensor_tensor(out=gm[:], in0=g_gt[:], in1=hmp[:],
                                        op=ALU.mult)
                rank = pp.tile([128, 1], F32, tag="rank")
                nc.vector.tensor_reduce(rank[:], gm[:], axis=AX.X, op=ALU.add)
                # dropflag_i = rank_i in rand_idx
                randb_ps = ps_c.tile([128, 6], F32, tag="c1")
                nc.tensor.matmul(randb_ps[:], ones1[:], rand_f1[:],
                                 start=True, stop=True)
                feq = wp.tile([128, 6], F32, tag="feq")
                nc.vector.tensor_scalar(out=feq[:], in0=randb_ps[:],
                                        scalar1=rank[:], scalar2=None,
                                        op0=ALU.is_equal)
                dropflag = pp.tile([128, 1], F32, tag="dropflag")
                nc.vector.tensor_reduce(dropflag[:], feq[:], axis=AX.X, op=ALU.add)

                # ---------------- dropmask over full N ----------------
                dftp = ps_c.tile([1, 128], F32, tag="c1")
                nc.tensor.transpose(dftp[:], dropflag[:], id_f32[:])
                dft = wp.tile([1, 128], F32, tag="dft")
                nc.vector.tensor_copy(dft[:], dftp[:])
                fbp = ps_c.tile([128, 128], F32, tag="c1")
                nc.tensor.matmul(fbp[:], ones1[:], dft[:], start=True, stop=True)
                fb_sb = pp.tile([128, 128], F32, tag="fbsb")
                nc.vector.tensor_copy(fb_sb[:], fbp[:])
                fb3 = fb_sb[:, 0:NC_CAND].rearrange("p (k c) -> p k c", k=K)
                dropmask = pp.tile([128, K, 64], F32, tag="dropmask")
                nc.vector.memset(dropmask[:], 0.0)
                for r in range(C):
                    mr = wp.tile([128, K, 64], F32, tag="mr")
                    nc.vector.tensor_scalar(out=mr[:], in0=candmap[:],
                                            scalar1=float(r + 1), scalar2=None,
                                            op0=ALU.is_equal)
                    nc.vector.tensor_tensor(
                        out=mr[:], in0=mr[:],
                        in1=fb3[:, :, r].unsqueeze(2).broadcast_to([128, K, 64]),
                        op=ALU.mult)
                    nc.vector.tensor_tensor(out=dropmask[:], in0=dropmask[:],
                                            in1=mr[:], op=ALU.add)
                dm_u8 = pp.tile([128, K, 64], mybir.dt.uint8, tag="dmu8")
                nc.vector.tensor_copy(dm_u8[:], dropmask[:])
                a_msk = pp.tile([128, K, 64], F32, tag="amsk")
                nc.vector.tensor_copy(a_msk[:], a_hier[:])
                nc.vector.copy_predicated(a_msk[:], dm_u8[:], neg_t[:])
                a_out_t = pp.tile([128, 64, K], F32, tag="aoutt")
                nc.vector.tensor_copy(a_out_t[:], a_msk[:].transpose([0, 2, 1]))
                nc.sync.dma_start(
                    a_ext.ap().rearrange("(p j) k -> p j k", p=128),
                    a_out_t[:])

                # ---------------- corrections + outputs ----------------
                expv = wp.tile([128, 1], F32, tag="expv")
                nc.scalar.activation(expv[:], av[:], AF.Exp)
                wc1 = wp.tile([128, 1], F32, tag="wc1")
                nc.vector.tensor_tensor(out=wc1[:], in0=dropflag[:], in1=expv[:],
                                        op=ALU.mult)
                wcand = wp.tile([128, K], BF16, tag="wcand")
                nc.vector.tensor_scalar(out=wcand[:], in0=headsel[:],
                                        scalar1=wc1[:], scalar2=None,
                                        op0=ALU.mult)
                hcb = wp.tile([128, DI], BF16, tag="hcb")
                nc.vector.tensor_copy(hcb[:], h_cand[:])
                corr_ps = ps_c.tile([5, DI], F32, tag="corrps")
                nc.tensor.matmul(corr_ps[:], wcand[:], hcb[:], start=True, stop=True)
                cs_ps = ps_c.tile([5, 1], F32, tag="csps")
                nc.tensor.matmul(cs_ps[:], wcand[:], ones128[:], start=True, stop=True)
                # S per head as [5,1]: transpose one row of s_all
                s15 = wp.tile([1, K], F32, tag="s15")
                nc.vector.tensor_copy(s15[:], s_all[0:1, :])
                s5p = ps_c.tile([5, 1], F32, tag="s5p")
                nc.tensor.transpose(s5p[:], s15[:], id_f32[0:1, 0:1])
                cs_sb = wp.tile([5, 1], F32, tag="cssb")
                nc.vector.tensor_copy(cs_sb[:], cs_ps[:])
                smask = wp.tile([5, 1], F32, tag="smask")
                nc.vector.tensor_tensor(out=smask[:], in0=s5p[:], in1=cs_sb[:],
                                        op=ALU.subtract)
                srec = wp.tile([5, 1], F32, tag="srec")
                nc.vector.reciprocal(srec[:], smask[:])
                corr_sb = wp.tile([5, DI], F32, tag="corrsb")
                nc.vector.tensor_copy(corr_sb[:], corr_ps[:])
                afm = wp.tile([5, DI], F32, tag="afm")
                nc.vector.tensor_tensor(out=afm[:], in0=afps[:], in1=corr_sb[:],
                                        op=ALU.subtract)
                afn = wp.tile([5, DI], F32, tag="afn")
                nc.vector.tensor_scalar(out=afn[:], in0=afm[:], scalar1=srec[:],
                                        scalar2=None, op0=ALU.mult)
                afntp = ps_c.tile([128, 4, 5], F32, tag="c1")
                for t in range(4):
                    nc.tensor.transpose(afntp[:, t, :],
                                        afn[:, t * 128:(t + 1) * 128],
                                        id_f32[0:5, 0:5])
                afnt = wp.tile([128, 4, 5], F32, tag="afnt")
                nc.vector.tensor_copy(afnt[:], afntp[:])
                br_ps = ps_c.tile([1, K * 2], F32, tag="outp")
                for k in range(K):
                    for t in range(4):
                        nc.tensor.matmul(br_ps[:, 2 * k:2 * k + 2],
                                         afnt[:, t, k:k + 1], wcls[:, k, t, :],
                                         start=(t == 0), stop=(t == 3))
                br_sb = wp.tile([1, K * 2], F32, tag="brsb")
                nc.vector.tensor_copy(br_sb[:], br_ps[:])
                nc.sync.dma_start(br_ext.ap().rearrange("k c -> (k c)").unsqueeze(0),
                                  br_sb[:])
                sft = wp.tile([128, 4], F32, tag="sft")
                nc.vector.tensor_reduce(sft[:], afnt[:], axis=AX.X, op=ALU.add)
                sfs = wp.tile([128, 4], F32, tag="sfs")
                nc.vector.tensor_scalar(out=sfs[:], in0=sft[:], scalar1=1.0 / K,
                                        scalar2=None, op0=ALU.mult)
                nc.sync.dma_start(sf_ext.ap().rearrange("o (b p) -> p (o b)", p=128),
                                  sfs[:])
                so_ps = ps_c.tile([1, 2], F32, tag="outp")
                for t in range(4):
                    nc.tensor.matmul(so_ps[:], sfs[:, t:t + 1], wsl[:, t, :],
                                     start=(t == 0), stop=(t == 3))
                so_sb = wp.tile([1, 2], F32, tag="sosb")
                nc.vector.tensor_copy(so_sb[:], so_ps[:])
                nc.sync.dma_start(so_ext[:], so_sb[:])
    nc.finalize()
    return nc


def kernel(**inputs):
    global _compiled
    x = np.ascontiguousarray(np.asarray(inputs["x"], dtype=np.float32))
    names = ["W_dr", "W_v", "W_u", "W_a", "W_cls", "W_slide"]
    w = {n: np.ascontiguousarray(np.asarray(inputs[n], np.float32)) for n in names}
    rand_idx = np.ascontiguousarray(np.asarray(inputs["rand_idx"], np.int32))

    if _compiled is None:
        _compiled = _build()
    nc = _compiled

    in_maps = []
    for b in range(B):
        m = {"x": x[b], "rand_idx": rand_idx}
        m.update(w)
        in_maps.append(m)
    res = run_bass_kernel_spmd(nc, in_maps, core_ids=list(range(B)),
                               trace=TRACE)
    if TRACE:
        print("HW exec time:", res.exec_time_ns, "ns")
        print("mean exec:", res.mean_exec_time_ns,
              "max core:", res.max_exec_time_core_id)

    A = np.stack([res.results[b]["A_out"] for b in range(B)])        # (B, N, K)
    br = np.stack([res.results[b]["br"] for b in range(B)], axis=1)  # (K, B, 2)
    so = np.concatenate([res.results[b]["so"] for b in range(B)])    # (B, 2)
    sf = np.concatenate([res.results[b]["sf"] for b in range(B)])    # (B, DI)
    return (br.astype(np.float32), so.astype(np.float32),
            A[None].astype(np.float32), sf.astype(np.float32))
